# revision 84
# baseline (speedup 1.0000x reference)
"""Trainium2 Bass kernel: multi-head attention with quantum (cumprod-of-cos) transform.

Full-input contract: kernel(**inputs) takes the unsharded inputs and returns the
full [B, S, E] output. Internally shards over 8 NeuronCores: data-parallel over
batch (B=2) x tensor-parallel over head-groups (4 heads per core).

Per-core pipeline (b = batch, g = head-group of 4 heads, EG = 256 e-dims):

Phase A (per 128-row s-tile, 16 tiles; per-tile theta psum is one
bank-padded tile per name -- matmul accumulation-group clears are
PSUM-bank-granular, so groups must not share a 2KB bank):
  theta [s, 256]x3 = x_tile (stationary) @ [Wq|Wk|Wv] slices (moving, fp16)
  c = sin(theta + pi/2) = cos(theta)          (3 ACT ops per tile, fp16 out)
  z = cumprod(c) along d (64 per head) via tensor_tensor_scan on DVE
      (op0=mult, op1=bypass, initial=1.0) -- one instruction per head
  zq/zk tiles [s, d] are PE-transposed ([128,128] fp16, identity-matmul) to
      zqT/zkT [d, s]; psum->sbuf copies ride ACT (Copy is in every table)
  vz tiles [t, 8, 64] interleave ones with z_v (slot order ones-first so the
      softmax denominator lands at psum partition 0, where the custom-DVE
      reciprocal reads correctly -- it misreads base_partition=64 inputs)

Phase B attention (two interleaved (head, s-chunk) streams so one stream's
scores->exp->acc latency hides behind the other's matmuls; s-chunk-major so
each chunk's final projection interleaves into the next chunk's stream):
  scoresT [t,s] = zkT t-tile (stationary) x zqT (moving), K=d=64, fp16,
      tile_position=(dbase,0): the two streams use disjoint PE row-group
      halves and run concurrently on hardware
  ex = exp(scores/8), fp16: 5/9 of tiles exact on ACT, 4/9 on DVE via the
      Schraudolph fp16 bit-trick (one tensor_scalar: i16 = A*s + B, bitcast)
  acc matmul: stationary [ones | vz] -> rows 0:64 = softmax denominator,
      rows 64:128 = unnormalized out^T (denominator free via ones columns)
  rec = reciprocal_approx_fast(denom) on DVE; oz = acc[64:128] * rec (fp16);
      rec/oz are deferred into the next group's t-slots to keep DVE smooth

Final: yT[e, s] partial = WcT slice (stationary) x oz (moving), fp16 matmul;
  psum->sbuf fp16 copies alternate DVE/ACT; DMA out per chunk mid-attention,
  batched 4-chunk DMAs for the last s-chunk's drain. Host sums 4 partials
  per batch.

ACT table loads: exactly 2 (Sin set for phase A, Exp set for phase B).
"""

import os
import sys

import numpy as np

if "/opt/trn_rl_repo" not in sys.path:
    sys.path.insert(0, "/opt/trn_rl_repo")

import concourse.bass as bass  # noqa: F401
import concourse.tile as tile
from concourse import bacc
from concourse import mybir
from concourse.bass_utils import run_bass_kernel_spmd

AF = mybir.ActivationFunctionType
ALU = mybir.AluOpType
F32 = mybir.dt.float32
F32R = mybir.dt.float32r
F16 = mybir.dt.float16
I16 = mybir.dt.int16

B, S, E, H, D = 2, 2048, 1024, 16, 64
NCORES = 8
HG = 4          # heads per core
EG = HG * D     # 256
P = 128
NT = S // P     # 16 s-tiles
KC = E // P     # 8 contraction tiles for the projections
HALF_PI = float(np.pi / 2)
INV_SQRT_D = 0.125  # 1/sqrt(64)

# fp16 Schraudolph: exp(s/8) ~= bitcast_f16(i16(EXP_A*s + EXP_B)); max rel
# err ~3.1% (applied to EXP_DVE of every EXP_MOD attention tiles).
EXP_A = 1024.0 * float(np.log2(np.e)) / 8.0
EXP_B = 15315.75
EXP_MOD = 9
EXP_DVE = (1, 3, 5, 7)   # which idx % EXP_MOD values go to the DVE bit-trick


def _attention(tc, oz_tiles, zqT, zkT, vzts, wc_t, yT, dbg=None):
    nc = tc.nc
    ex_idx = 0
    cp_idx = 0
    # Two independent (head, s-chunk) streams are interleaved so one stream's
    # scores->exp->accumulate latency chain hides behind the other stream's
    # matmuls in the PE FIFO. Groups run s-chunk-major so a chunk's final
    # projection can interleave into the next chunk's attention stream.
    combos = [(m, h2, sb) for sb in range(4) for m in range(2)
              for h2 in range(2)]
    pending = []  # (mo, sb) final-projection chunks whose oz slices are done

    with (
        tc.tile_pool(name="y", bufs=3) as yp,
        tc.tile_pool(name="norm", bufs=2) as nrm,
    ):
        def emit_final_chunk(pool, bufs):
            nonlocal cp_idx
            mo, sb = pending.pop(0)
            ssl = slice(sb * 512, (sb + 1) * 512)
            py = pool.tile([P, 512], F32, tag="py", bufs=bufs,
                           name=f"py{mo}_{sb}")
            for kk in range(2):
                nc.tensor.matmul(
                    py[:],
                    lhsT=wc_t[:, kk, mo * P:(mo + 1) * P],
                    rhs=oz_tiles[kk][:, ssl],
                    start=(kk == 0), stop=(kk == 1),
                )
            yt = yp.tile([P, 512], F16, tag="y", name=f"yt{mo}_{sb}")
            if cp_idx % 2 == 0:
                nc.vector.tensor_copy(out=yt[:], in_=py[:])
            else:
                nc.scalar.copy(out=yt[:], in_=py[:])
            cp_idx += 1
            nc.sync.dma_start(out=yT[mo * P:(mo + 1) * P, ssl], in_=yt[:])

        with (
            tc.tile_pool(name="psB", bufs=1, space="PSUM") as psB,
            tc.tile_pool(name="exps", bufs=10) as exq,
        ):
            def make_norm_ops(accs, pair, pi):
                # rec + oz for a finished pair, emitted one per t-slot of the
                # NEXT group so the DVE burst never delays that group's exps
                ops = []
                for s, (m, h2, sb) in enumerate(pair):
                    dbase = h2 * D
                    ssl = slice(sb * 512, (sb + 1) * 512)

                    def _rec(s=s, pi=pi):
                        rec = nrm.tile([D, 512], F32, tag=f"rec{s}",
                                       name=f"rec{pi}_{s}")
                        nc.vector.reciprocal_approx_fast(
                            rec[:], accs[s][0:D, :])
                        recs[s] = rec

                    def _oz(s=s, m=m, dbase=dbase, ssl=ssl, pair=pair):
                        nc.vector.tensor_tensor(
                            out=oz_tiles[m][dbase:dbase + D, ssl],
                            in0=accs[s][D:2 * D, :], in1=recs[s][:],
                            op=ALU.mult,
                        )
                        if s == 1 and pair[-1][0] == 1:
                            sb = pair[-1][2]
                            pending.extend(
                                (mo, sb) for mo in range(E // P))

                    ops.append(_rec)
                    ops.append(_oz)
                # order: rec0, rec1, oz0, oz1
                return [ops[0], ops[2], ops[1], ops[3]]

            recs = [None, None]
            deferred = []
            for pi in range(0, len(combos), 2):
                pair = combos[pi:pi + 2]
                accs = []
                for s, (m, h2, sb) in enumerate(pair):
                    accs.append(psB.tile([P, 512], F32, tag=f"acc{s}", bufs=2,
                                         name=f"acc{pi}_{s}"))
                exs = [[None] * NT for _ in pair]
                nfin = 0
                for t in range(NT + 2):
                    for s, (m, h2, sb) in enumerate(pair):
                        h = 2 * m + h2
                        dbase = h2 * D
                        ssl = slice(sb * 512, (sb + 1) * 512)
                        if t < NT:
                            sc = psB.tile([P, 512], F32, tag="sc", bufs=3,
                                          name=f"sc{pi}_{s}_{t}")
                            # K=64: the pair's two streams use disjoint PE
                            # row-group halves and run concurrently on HW
                            nc.tensor.matmul(
                                sc[:],
                                lhsT=zkT[m][dbase:dbase + D,
                                            t * P:(t + 1) * P],
                                rhs=zqT[m][dbase:dbase + D, ssl],
                                start=True, stop=True,
                                tile_position=(dbase, 0),
                            )
                            ex = exq.tile([P, 512], F16, tag="ex",
                                          name=f"ex{pi}_{s}_{t}")
                            if (ex_idx % EXP_MOD) in EXP_DVE:
                                nc.vector.tensor_scalar(
                                    out=ex[:].bitcast(I16), in0=sc[:],
                                    scalar1=EXP_A, scalar2=EXP_B,
                                    op0=ALU.mult, op1=ALU.add,
                                )
                            else:
                                nc.scalar.activation(
                                    ex[:], sc[:], AF.Exp, scale=INV_SQRT_D)
                            if _DEBUG and pi == 0 and s == 0 and t == 0:
                                nc.sync.dma_start(
                                    out=dbg["dbg_ex0"][:], in_=ex[:])
                            ex_idx += 1
                            exs[s][t] = ex
                        if t > 1:
                            tp = t - 2
                            nc.tensor.matmul(
                                accs[s][:],
                                lhsT=vzts[tp][:, 2 * h:2 * h + 2, :].rearrange(
                                    "p a d -> p (a d)"),
                                rhs=exs[s][tp][:],
                                start=(tp == 0), stop=(tp == NT - 1),
                            )
                    # one deferred rec/oz (prev group) or one final-projection
                    # chunk per t slot: the interleave gives each py's copy a
                    # full iteration to drain, so py bufs=1 never
                    # head-of-line-blocks the PE FIFO
                    if deferred and t >= 2:
                        deferred.pop(0)()
                    elif pending and nfin < 5:
                        # cap per-group final chunks so a few are left to
                        # fill the next group's tail while its rec/oz flush
                        emit_final_chunk(psB, 1)
                        nfin += 1
                deferred.extend(make_norm_ops(accs, pair, pi))

            # flush the last group's rec/oz while its acc pool is still open
            for op in deferred:
                op()

            # Drain inside psB (no pool-open barrier): the last s-chunk's 8
            # final-projection chunks reuse the freed sc (3 bufs) + py (1)
            # banks, rotating at most 4 open accumulations. The first 4 kk=0
            # matmuls depend only on oz[0] and run while the rec/oz flush
            # above occupies DVE; kk=1 + copies chase; two batched output
            # DMAs (one descriptor per 4 chunks instead of 8 at ~625 ns
            # HWDGE slot each).
            last_sb = combos[-1][2]
            chunks = list(pending)
            assert chunks == [(mo, last_sb) for mo in range(E // P)], chunks
            pending.clear()
            ssl3 = slice(last_sb * 512, (last_sb + 1) * 512)
            dpys = {}

            def drain_mm0(i):
                mo, _ = chunks[i]
                tag, nb = ("sc", 3) if i % 4 != 3 else ("py", 1)
                py = psB.tile([P, 512], F32, tag=tag, bufs=nb,
                              name=f"pyd{i}")
                dpys[i] = py
                nc.tensor.matmul(
                    py[:], lhsT=wc_t[:, 0, mo * P:(mo + 1) * P],
                    rhs=oz_tiles[0][:, ssl3], start=True, stop=False)

            ytb = yp.tile([P, len(chunks), 512], F16, tag="ytb",
                          bufs=1, name="ytb")
            for i in range(4):
                drain_mm0(i)
            # DMA batches 3+3+2: each is issued right after its chunks'
            # copies; the transfers serialize on the DMA engine, so a small
            # final piece shortens the end-of-kernel critical path
            splits = [(0, 3), (3, 6), (6, 7), (7, 8)]
            for i in range(len(chunks)):
                mo, _ = chunks[i]
                nc.tensor.matmul(
                    dpys[i][:], lhsT=wc_t[:, 1, mo * P:(mo + 1) * P],
                    rhs=oz_tiles[1][:, ssl3], start=False, stop=True)
                if i % 2 == 0:
                    nc.vector.tensor_copy(out=ytb[:, i, :], in_=dpys[i][:])
                else:
                    nc.scalar.copy(out=ytb[:, i, :], in_=dpys[i][:])
                if i + 4 < len(chunks):
                    drain_mm0(i + 4)
                for lo, hi in splits:
                    if i + 1 == hi:
                        mo0 = chunks[lo][0]
                        nc.sync.dma_start(
                            out=yT[mo0 * P:(mo0 + hi - lo) * P,
                                   ssl3].rearrange("(k p) s -> p k s", p=P),
                            in_=ytb[:, lo:hi, :])


_DEBUG = bool(int(os.environ.get("QK_DEBUG", "0")))


def _build_body(tc, xT, wT, wcT, ident, yT, dbg=None):
    nc = tc.nc

    with (
        tc.tile_pool(name="const", bufs=1) as const,
        tc.tile_pool(name="wc", bufs=1) as wcp,
        tc.tile_pool(name="vz", bufs=1) as vzp,
        tc.tile_pool(name="zT", bufs=1) as zTp,
    ):
        hp = const.tile([P, 1], F32)
        nc.vector.memset(hp[:], HALF_PI)
        ones = const.tile([P, D], F16)
        nc.vector.memset(ones[:], 1.0)
        id_t = const.tile([P, P], F16)
        wc_t = wcp.tile([P, 2, E], F16)

        vzts = [
            vzp.tile([P, 8, D], F16, tag=f"vz{t}", name=f"vz{t}")
            for t in range(NT)
        ]
        # slot order per head: (ones, vz) so the acc-matmul puts the softmax
        # denominator in out rows 0:64 — reciprocal_approx_fast (custom DVE
        # op) misreads PSUM inputs with nonzero base partition, so the
        # denominator must sit at partition 0
        for t in range(NT):
            nc.gpsimd.memset(vzts[t][:, 0:8:2, :], 1.0)

        zqT = [zTp.tile([P, S], F16, tag=f"zqT{m}", name=f"zqT{m}")
               for m in range(2)]
        zkT = [zTp.tile([P, S], F16, tag=f"zkT{m}", name=f"zkT{m}")
               for m in range(2)]

        # ---------------- Phase A ----------------
        with (
            tc.tile_pool(name="psA", bufs=1, space="PSUM") as psA,
            tc.tile_pool(name="psT", bufs=1, space="PSUM") as psT,
            tc.tile_pool(name="x", bufs=KC) as xp,
            tc.tile_pool(name="w", bufs=1) as wp,
            tc.tile_pool(name="c", bufs=4) as cp,
            tc.tile_pool(name="zs", bufs=1) as zsp,
        ):
            # x arrives s-chunk-major so the first theta psum completes after
            # ~1/4 of the x bytes instead of all of them; w arrives per
            # k-chunk interleaved with the first x chunk so matmul k=0 can
            # start after ~0.5 MB of DMA
            w_t = wp.tile([P, KC, 3 * EG], F16, tag="w")
            wTr = wT.rearrange("(k p) n -> p k n", p=P)
            xts = [xp.tile([P, S], F16, tag="x", name=f"x{k}") for k in range(KC)]
            for sb in range(4):
                for k in range(KC):
                    if sb == 0:
                        if k == 0:
                            # the very first theta matmul needs only x0's
                            # first 128 cols + w0: land those two descriptors
                            # first so the PE starts ~1 us earlier
                            nc.sync.dma_start(out=xts[0][:, 0:P],
                                              in_=xT[0:P, 0:P])
                        nc.sync.dma_start(out=w_t[:, k, :], in_=wTr[:, k, :])
                    lo = P if (sb == 0 and k == 0) else sb * 512
                    nc.sync.dma_start(
                        out=xts[k][:, lo:(sb + 1) * 512],
                        in_=xT[k * P:(k + 1) * P, lo:(sb + 1) * 512],
                    )
                if sb == 0:
                    nc.sync.dma_start(out=id_t[:], in_=ident[:])
                    nc.sync.dma_start(
                        out=wc_t[:],
                        in_=wcT.rearrange("(k p) e -> p k e", p=P))

            trans_q = []  # pipelined transposes: emit for tile t-1 during t
            for t in range(NT + 1):
                if t < NT:
                    tsl = slice(t * P, (t + 1) * P)
                    # one bank-padded psum tile per name: a matmul accumulation
                    # group's start-clear is bank-granular, so groups must not
                    # share a 2KB PSUM bank
                    # q+k share one full-bank [128,512] accumulation region
                    # (one matmul group per bank); v gets its own padded bank
                    th_qk = psA.tile([P, 2 * EG], F32, tag="thqk", bufs=3,
                                     name=f"thqk_{t}")
                    th_v = psA.tile([P, EG], F32, tag="thv", bufs=3,
                                    padded_shape=[P, 512], name=f"thv_{t}")
                    for k in range(KC):
                        nc.tensor.matmul(
                            th_qk[:],
                            lhsT=xts[k][:, tsl],
                            rhs=w_t[:, k, 0:2 * EG],
                            start=(k == 0), stop=(k == KC - 1),
                        )
                        nc.tensor.matmul(
                            th_v[:],
                            lhsT=xts[k][:, tsl],
                            rhs=w_t[:, k, 2 * EG:3 * EG],
                            start=(k == 0), stop=(k == KC - 1),
                        )
                    c = cp.tile([P, 3 * EG], F16, tag="c", name=f"c{t}")
                    nc.scalar.activation(
                        c[:, 0:2 * EG], th_qk[:], AF.Sin, bias=hp[:])
                    nc.scalar.activation(
                        c[:, 2 * EG:3 * EG], th_v[:], AF.Sin, bias=hp[:])
                    zq_s = zsp.tile([P, EG], F16, tag="zq", bufs=3,
                                    name=f"zqs{t}")
                    zk_s = zsp.tile([P, EG], F16, tag="zk", bufs=3,
                                    name=f"zks{t}")
                    # q,k scans first: they feed the transposes (critical
                    # path); v scans are only consumed in phase B
                    for h in range(HG):
                        dsl = slice(h * D, (h + 1) * D)
                        nc.vector.tensor_tensor_scan(
                            zq_s[:, dsl], c[:, dsl], ones[:], 1.0,
                            ALU.mult, ALU.bypass)
                        nc.vector.tensor_tensor_scan(
                            zk_s[:, dsl], c[:, EG + h * D:EG + (h + 1) * D],
                            ones[:], 1.0, ALU.mult, ALU.bypass)
                    for h in range(HG):
                        nc.vector.tensor_tensor_scan(
                            vzts[t][:, 2 * h + 1, :],
                            c[:, 2 * EG + h * D:2 * EG + (h + 1) * D],
                            ones[:], 1.0, ALU.mult, ALU.bypass)
                    if _DEBUG and t == 0:
                        nc.sync.dma_start(out=dbg["dbg_c0"][:], in_=c[:])
                        nc.sync.dma_start(out=dbg["dbg_zqs0"][:], in_=zq_s[:])
                        nc.sync.dma_start(out=dbg["dbg_zks0"][:], in_=zk_s[:])
                    trans_q.append((t, zq_s, zk_s))
                if t > 0:
                    tp, zq_s, zk_s = trans_q[t - 1]
                    tsl = slice(tp * P, (tp + 1) * P)
                    for m in range(2):
                        msl = slice(m * P, (m + 1) * P)
                        for ci, (src, dst) in enumerate(
                                ((zq_s, zqT[m]), (zk_s, zkT[m]))):
                            pt = psT.tile([P, P], F16, tag="pt", bufs=2,
                                          padded_shape=[P, 1024],
                                          name=f"pt{tp}{m}")
                            nc.tensor.transpose(pt[:], src[:, msl], id_t[:])
                            # GPSIMD cannot read PSUM; Copy is in every ACT
                            # table so it costs no table switch. The last two
                            # tiles' copies gate the phase A->B pool handoff:
                            # split them across both engines to halve that
                            # serial chain.
                            if tp >= NT - 2 and (2 * m + ci) % 2 == 1:
                                nc.vector.tensor_copy(out=dst[:, tsl],
                                                      in_=pt[:])
                            else:
                                nc.scalar.copy(out=dst[:, tsl], in_=pt[:])

        if _DEBUG:
            nc.sync.dma_start(out=dbg["dbg_zqT0"][:], in_=zqT[0][:])
            nc.sync.dma_start(out=dbg["dbg_zkT0"][:], in_=zkT[0][:])
            nc.sync.dma_start(
                out=dbg["dbg_vz0"][:],
                in_=vzts[0][:].rearrange("p a d -> p (a d)"))

        # ---------------- Phase B (attention + fused final projection) ----
        with tc.tile_pool(name="oz", bufs=1) as ozp:
            oz_tiles = [ozp.tile([P, S], F16, tag=f"oz{m}", name=f"oz{m}")
                        for m in range(2)]
            _attention(tc, oz_tiles, zqT, zkT, vzts, wc_t, yT, dbg)
            if _DEBUG:
                nc.sync.dma_start(out=dbg["dbg_oz0"][:], in_=oz_tiles[0][:])


def build_bass():
    nc = bacc.Bacc(None, target_bir_lowering=False)
    xT = nc.dram_tensor("xT", [E, S], F16, kind="ExternalInput")
    wT = nc.dram_tensor("wT", [E, 3 * EG], F16, kind="ExternalInput")
    wcT = nc.dram_tensor("wcT", [EG, E], F16, kind="ExternalInput")
    ident = nc.dram_tensor("ident", [P, P], F16, kind="ExternalInput")
    yT = nc.dram_tensor("yT", [E, S], F16, kind="ExternalOutput")
    dbg = {}
    if _DEBUG:
        for nm, shp, dt in (("dbg_zqT0", [P, S], F16), ("dbg_zkT0", [P, S], F16),
                            ("dbg_vz0", [P, 8 * D], F16),
                            ("dbg_oz0", [P, S], F16),
                            ("dbg_c0", [P, 3 * EG], F16),
                            ("dbg_zqs0", [P, EG], F16),
                            ("dbg_zks0", [P, EG], F16),
                            ("dbg_ex0", [P, 512], F16)):
            dbg[nm] = nc.dram_tensor(nm, shp, dt, kind="ExternalOutput")[:]
    with tile.TileContext(nc) as tc:
        _build_body(tc, xT[:], wT[:], wcT[:], ident[:], yT[:], dbg)
    nc.finalize()
    return nc


_NC_CACHE = None


def _get_nc():
    global _NC_CACHE
    if _NC_CACHE is None:
        _NC_CACHE = build_bass()
    return _NC_CACHE


def kernel(x, Wq, Wk, Wv, Wc, bc, **kw):
    x = np.asarray(x, np.float32)
    ident = np.eye(P, dtype=np.float16)
    in_maps = []
    for c in range(NCORES):
        b, g = divmod(c, NCORES // B)
        sl = slice(g * EG, (g + 1) * EG)
        wqkv = np.concatenate(
            [np.asarray(Wq)[sl, :].T, np.asarray(Wk)[sl, :].T,
             np.asarray(Wv)[sl, :].T], axis=1).astype(np.float16)
        in_maps.append({
            "xT": np.ascontiguousarray(np.asarray(x[b]).T.astype(np.float16)),
            "wT": np.ascontiguousarray(wqkv),
            "wcT": np.ascontiguousarray(
                np.asarray(Wc)[:, sl].T.astype(np.float16)),
            "ident": ident,
        })
    nc = _get_nc()
    res = run_bass_kernel_spmd(
        nc, in_maps, core_ids=list(range(NCORES)),
        trace=bool(int(os.environ.get("QK_TRACE", "0"))),
    )
    y = np.zeros((B, S, E), np.float32)
    for c in range(NCORES):
        b = c // (NCORES // B)
        y[b] += res.results[c]["yT"].astype(np.float32).T
    y += np.asarray(bc, np.float32)
    globals()["_LAST_RESULT"] = res
    return y


# revision 85
# speedup vs baseline: 1.0003x; 1.0003x over previous
"""Trainium2 Bass kernel: multi-head attention with quantum (cumprod-of-cos) transform.

Full-input contract: kernel(**inputs) takes the unsharded inputs and returns the
full [B, S, E] output. Internally shards over 8 NeuronCores: data-parallel over
batch (B=2) x tensor-parallel over head-groups (4 heads per core).

Per-core pipeline (b = batch, g = head-group of 4 heads, EG = 256 e-dims):

Phase A (per 128-row s-tile, 16 tiles; per-tile theta psum is one
bank-padded tile per name -- matmul accumulation-group clears are
PSUM-bank-granular, so groups must not share a 2KB bank):
  theta [s, 256]x3 = x_tile (stationary) @ [Wq|Wk|Wv] slices (moving, fp16)
  c = sin(theta + pi/2) = cos(theta)          (3 ACT ops per tile, fp16 out)
  z = cumprod(c) along d (64 per head) via tensor_tensor_scan on DVE
      (op0=mult, op1=bypass, initial=1.0) -- one instruction per head
  zq/zk tiles [s, d] are PE-transposed ([128,128] fp16, identity-matmul) to
      zqT/zkT [d, s]; psum->sbuf copies ride ACT (Copy is in every table)
  vz tiles [t, 8, 64] interleave ones with z_v (slot order ones-first so the
      softmax denominator lands at psum partition 0, where the custom-DVE
      reciprocal reads correctly -- it misreads base_partition=64 inputs)

Phase B attention (two interleaved (head, s-chunk) streams so one stream's
scores->exp->acc latency hides behind the other's matmuls; s-chunk-major so
each chunk's final projection interleaves into the next chunk's stream):
  scoresT [t,s] = zkT t-tile (stationary) x zqT (moving), K=d=64, fp16,
      tile_position=(dbase,0): the two streams use disjoint PE row-group
      halves and run concurrently on hardware
  ex = exp(scores/8), fp16: 5/9 of tiles exact on ACT, 4/9 on DVE via the
      Schraudolph fp16 bit-trick (one tensor_scalar: i16 = A*s + B, bitcast)
  acc matmul: stationary [ones | vz] -> rows 0:64 = softmax denominator,
      rows 64:128 = unnormalized out^T (denominator free via ones columns)
  rec = reciprocal_approx_fast(denom) on DVE; oz = acc[64:128] * rec (fp16);
      rec/oz are deferred into the next group's t-slots to keep DVE smooth

Final: yT[e, s] partial = WcT slice (stationary) x oz (moving), fp16 matmul;
  psum->sbuf fp16 copies alternate DVE/ACT; DMA out per chunk mid-attention,
  batched 4-chunk DMAs for the last s-chunk's drain. Host sums 4 partials
  per batch.

ACT table loads: exactly 2 (Sin set for phase A, Exp set for phase B).
"""

import os
import sys

import numpy as np

if "/opt/trn_rl_repo" not in sys.path:
    sys.path.insert(0, "/opt/trn_rl_repo")

import concourse.bass as bass  # noqa: F401
import concourse.tile as tile
from concourse import bacc
from concourse import mybir
from concourse.bass_utils import run_bass_kernel_spmd

AF = mybir.ActivationFunctionType
ALU = mybir.AluOpType
F32 = mybir.dt.float32
F32R = mybir.dt.float32r
F16 = mybir.dt.float16
I16 = mybir.dt.int16

B, S, E, H, D = 2, 2048, 1024, 16, 64
NCORES = 8
HG = 4          # heads per core
EG = HG * D     # 256
P = 128
NT = S // P     # 16 s-tiles
KC = E // P     # 8 contraction tiles for the projections
HALF_PI = float(np.pi / 2)
INV_SQRT_D = 0.125  # 1/sqrt(64)

# fp16 Schraudolph: exp(s/8) ~= bitcast_f16(i16(EXP_A*s + EXP_B)); max rel
# err ~3.1% (applied to EXP_DVE of every EXP_MOD attention tiles).
EXP_A = 1024.0 * float(np.log2(np.e)) / 8.0
EXP_B = 15315.75
EXP_MOD = 9
EXP_DVE = (1, 3, 5, 7)   # which idx % EXP_MOD values go to the DVE bit-trick


def _attention(tc, oz_tiles, zqT, zkT, vzts, wc_t, yT, dbg=None):
    nc = tc.nc
    ex_idx = 0
    cp_idx = 0
    # Two independent (head, s-chunk) streams are interleaved so one stream's
    # scores->exp->accumulate latency chain hides behind the other stream's
    # matmuls in the PE FIFO. Groups run s-chunk-major so a chunk's final
    # projection can interleave into the next chunk's attention stream.
    combos = [(m, h2, sb) for sb in range(4) for m in range(2)
              for h2 in range(2)]
    pending = []  # (mo, sb) final-projection chunks whose oz slices are done

    with (
        tc.tile_pool(name="y", bufs=3) as yp,
        tc.tile_pool(name="norm", bufs=2) as nrm,
    ):
        def emit_final_chunk(pool, bufs):
            nonlocal cp_idx
            mo, sb = pending.pop(0)
            ssl = slice(sb * 512, (sb + 1) * 512)
            py = pool.tile([P, 512], F32, tag="py", bufs=bufs,
                           name=f"py{mo}_{sb}")
            for kk in range(2):
                nc.tensor.matmul(
                    py[:],
                    lhsT=wc_t[:, kk, mo * P:(mo + 1) * P],
                    rhs=oz_tiles[kk][:, ssl],
                    start=(kk == 0), stop=(kk == 1),
                )
            yt = yp.tile([P, 512], F16, tag="y", name=f"yt{mo}_{sb}")
            if cp_idx % 2 == 0:
                nc.vector.tensor_copy(out=yt[:], in_=py[:])
            else:
                nc.scalar.copy(out=yt[:], in_=py[:])
            cp_idx += 1
            nc.sync.dma_start(out=yT[mo * P:(mo + 1) * P, ssl], in_=yt[:])

        with (
            tc.tile_pool(name="psB", bufs=1, space="PSUM") as psB,
            tc.tile_pool(name="exps", bufs=10) as exq,
        ):
            def make_norm_ops(accs, pair, pi):
                # rec + oz for a finished pair, emitted one per t-slot of the
                # NEXT group so the DVE burst never delays that group's exps
                ops = []
                for s, (m, h2, sb) in enumerate(pair):
                    dbase = h2 * D
                    ssl = slice(sb * 512, (sb + 1) * 512)

                    def _rec(s=s, pi=pi):
                        rec = nrm.tile([D, 512], F32, tag=f"rec{s}",
                                       name=f"rec{pi}_{s}")
                        nc.vector.reciprocal_approx_fast(
                            rec[:], accs[s][0:D, :])
                        recs[s] = rec

                    def _oz(s=s, m=m, dbase=dbase, ssl=ssl, pair=pair):
                        nc.vector.tensor_tensor(
                            out=oz_tiles[m][dbase:dbase + D, ssl],
                            in0=accs[s][D:2 * D, :], in1=recs[s][:],
                            op=ALU.mult,
                        )
                        if s == 1 and pair[-1][0] == 1:
                            sb = pair[-1][2]
                            pending.extend(
                                (mo, sb) for mo in range(E // P))

                    ops.append(_rec)
                    ops.append(_oz)
                # order: rec0, rec1, oz0, oz1
                return [ops[0], ops[2], ops[1], ops[3]]

            recs = [None, None]
            deferred = []
            for pi in range(0, len(combos), 2):
                pair = combos[pi:pi + 2]
                accs = []
                for s, (m, h2, sb) in enumerate(pair):
                    accs.append(psB.tile([P, 512], F32, tag=f"acc{s}", bufs=2,
                                         name=f"acc{pi}_{s}"))
                exs = [[None] * NT for _ in pair]
                nfin = 0
                for t in range(NT + 2):
                    for s, (m, h2, sb) in enumerate(pair):
                        h = 2 * m + h2
                        dbase = h2 * D
                        ssl = slice(sb * 512, (sb + 1) * 512)
                        if t < NT:
                            sc = psB.tile([P, 512], F32, tag="sc", bufs=3,
                                          name=f"sc{pi}_{s}_{t}")
                            # K=64: the pair's two streams use disjoint PE
                            # row-group halves and run concurrently on HW
                            nc.tensor.matmul(
                                sc[:],
                                lhsT=zkT[m][dbase:dbase + D,
                                            t * P:(t + 1) * P],
                                rhs=zqT[m][dbase:dbase + D, ssl],
                                start=True, stop=True,
                                tile_position=(dbase, 0),
                            )
                            ex = exq.tile([P, 512], F16, tag="ex",
                                          name=f"ex{pi}_{s}_{t}")
                            if (ex_idx % EXP_MOD) in EXP_DVE:
                                nc.vector.tensor_scalar(
                                    out=ex[:].bitcast(I16), in0=sc[:],
                                    scalar1=EXP_A, scalar2=EXP_B,
                                    op0=ALU.mult, op1=ALU.add,
                                )
                            else:
                                nc.scalar.activation(
                                    ex[:], sc[:], AF.Exp, scale=INV_SQRT_D)
                            if _DEBUG and pi == 0 and s == 0 and t == 0:
                                nc.sync.dma_start(
                                    out=dbg["dbg_ex0"][:], in_=ex[:])
                            ex_idx += 1
                            exs[s][t] = ex
                        if t > 1:
                            tp = t - 2
                            nc.tensor.matmul(
                                accs[s][:],
                                lhsT=vzts[tp][:, 2 * h:2 * h + 2, :].rearrange(
                                    "p a d -> p (a d)"),
                                rhs=exs[s][tp][:],
                                start=(tp == 0), stop=(tp == NT - 1),
                            )
                    # one deferred rec/oz (prev group) or one final-projection
                    # chunk per t slot: the interleave gives each py's copy a
                    # full iteration to drain, so py bufs=1 never
                    # head-of-line-blocks the PE FIFO
                    if deferred and t >= 1:
                        deferred.pop(0)()
                    elif pending and nfin < 5:
                        # cap per-group final chunks so a few are left to
                        # fill the next group's tail while its rec/oz flush
                        emit_final_chunk(psB, 1)
                        nfin += 1
                deferred.extend(make_norm_ops(accs, pair, pi))

            # flush the last group's rec/oz while its acc pool is still open
            for op in deferred:
                op()

            # Drain inside psB (no pool-open barrier): the last s-chunk's 8
            # final-projection chunks reuse the freed sc (3 bufs) + py (1)
            # banks, rotating at most 4 open accumulations. The first 4 kk=0
            # matmuls depend only on oz[0] and run while the rec/oz flush
            # above occupies DVE; kk=1 + copies chase; two batched output
            # DMAs (one descriptor per 4 chunks instead of 8 at ~625 ns
            # HWDGE slot each).
            last_sb = combos[-1][2]
            chunks = list(pending)
            assert chunks == [(mo, last_sb) for mo in range(E // P)], chunks
            pending.clear()
            ssl3 = slice(last_sb * 512, (last_sb + 1) * 512)
            dpys = {}

            def drain_mm0(i):
                mo, _ = chunks[i]
                tag, nb = ("sc", 3) if i % 4 != 3 else ("py", 1)
                py = psB.tile([P, 512], F32, tag=tag, bufs=nb,
                              name=f"pyd{i}")
                dpys[i] = py
                nc.tensor.matmul(
                    py[:], lhsT=wc_t[:, 0, mo * P:(mo + 1) * P],
                    rhs=oz_tiles[0][:, ssl3], start=True, stop=False)

            ytb = yp.tile([P, len(chunks), 512], F16, tag="ytb",
                          bufs=1, name="ytb")
            for i in range(4):
                drain_mm0(i)
            # DMA batches 3+3+2: each is issued right after its chunks'
            # copies; the transfers serialize on the DMA engine, so a small
            # final piece shortens the end-of-kernel critical path
            splits = [(0, 3), (3, 6), (6, 7), (7, 8)]
            for i in range(len(chunks)):
                mo, _ = chunks[i]
                nc.tensor.matmul(
                    dpys[i][:], lhsT=wc_t[:, 1, mo * P:(mo + 1) * P],
                    rhs=oz_tiles[1][:, ssl3], start=False, stop=True)
                if i % 2 == 0:
                    nc.vector.tensor_copy(out=ytb[:, i, :], in_=dpys[i][:])
                else:
                    nc.scalar.copy(out=ytb[:, i, :], in_=dpys[i][:])
                if i + 4 < len(chunks):
                    drain_mm0(i + 4)
                for lo, hi in splits:
                    if i + 1 == hi:
                        mo0 = chunks[lo][0]
                        nc.sync.dma_start(
                            out=yT[mo0 * P:(mo0 + hi - lo) * P,
                                   ssl3].rearrange("(k p) s -> p k s", p=P),
                            in_=ytb[:, lo:hi, :])


_DEBUG = bool(int(os.environ.get("QK_DEBUG", "0")))


def _build_body(tc, xT, wT, wcT, ident, yT, dbg=None):
    nc = tc.nc

    with (
        tc.tile_pool(name="const", bufs=1) as const,
        tc.tile_pool(name="wc", bufs=1) as wcp,
        tc.tile_pool(name="vz", bufs=1) as vzp,
        tc.tile_pool(name="zT", bufs=1) as zTp,
    ):
        hp = const.tile([P, 1], F32)
        nc.vector.memset(hp[:], HALF_PI)
        ones = const.tile([P, D], F16)
        nc.vector.memset(ones[:], 1.0)
        id_t = const.tile([P, P], F16)
        wc_t = wcp.tile([P, 2, E], F16)

        vzts = [
            vzp.tile([P, 8, D], F16, tag=f"vz{t}", name=f"vz{t}")
            for t in range(NT)
        ]
        # slot order per head: (ones, vz) so the acc-matmul puts the softmax
        # denominator in out rows 0:64 — reciprocal_approx_fast (custom DVE
        # op) misreads PSUM inputs with nonzero base partition, so the
        # denominator must sit at partition 0
        for t in range(NT):
            nc.gpsimd.memset(vzts[t][:, 0:8:2, :], 1.0)

        zqT = [zTp.tile([P, S], F16, tag=f"zqT{m}", name=f"zqT{m}")
               for m in range(2)]
        zkT = [zTp.tile([P, S], F16, tag=f"zkT{m}", name=f"zkT{m}")
               for m in range(2)]

        # ---------------- Phase A ----------------
        with (
            tc.tile_pool(name="psA", bufs=1, space="PSUM") as psA,
            tc.tile_pool(name="psT", bufs=1, space="PSUM") as psT,
            tc.tile_pool(name="x", bufs=KC) as xp,
            tc.tile_pool(name="w", bufs=1) as wp,
            tc.tile_pool(name="c", bufs=4) as cp,
            tc.tile_pool(name="zs", bufs=1) as zsp,
        ):
            # x arrives s-chunk-major so the first theta psum completes after
            # ~1/4 of the x bytes instead of all of them; w arrives per
            # k-chunk interleaved with the first x chunk so matmul k=0 can
            # start after ~0.5 MB of DMA
            w_t = wp.tile([P, KC, 3 * EG], F16, tag="w")
            wTr = wT.rearrange("(k p) n -> p k n", p=P)
            xts = [xp.tile([P, S], F16, tag="x", name=f"x{k}") for k in range(KC)]
            for sb in range(4):
                for k in range(KC):
                    if sb == 0:
                        if k == 0:
                            # the very first theta matmul needs only x0's
                            # first 128 cols + w0: land those two descriptors
                            # first so the PE starts ~1 us earlier
                            nc.sync.dma_start(out=xts[0][:, 0:P],
                                              in_=xT[0:P, 0:P])
                        nc.sync.dma_start(out=w_t[:, k, :], in_=wTr[:, k, :])
                    lo = P if (sb == 0 and k == 0) else sb * 512
                    nc.sync.dma_start(
                        out=xts[k][:, lo:(sb + 1) * 512],
                        in_=xT[k * P:(k + 1) * P, lo:(sb + 1) * 512],
                    )
                if sb == 0:
                    nc.sync.dma_start(out=id_t[:], in_=ident[:])
                    nc.sync.dma_start(
                        out=wc_t[:],
                        in_=wcT.rearrange("(k p) e -> p k e", p=P))

            trans_q = []  # pipelined transposes: emit for tile t-1 during t
            for t in range(NT + 1):
                if t < NT:
                    tsl = slice(t * P, (t + 1) * P)
                    # one bank-padded psum tile per name: a matmul accumulation
                    # group's start-clear is bank-granular, so groups must not
                    # share a 2KB PSUM bank
                    # q+k share one full-bank [128,512] accumulation region
                    # (one matmul group per bank); v gets its own padded bank
                    th_qk = psA.tile([P, 2 * EG], F32, tag="thqk", bufs=3,
                                     name=f"thqk_{t}")
                    th_v = psA.tile([P, EG], F32, tag="thv", bufs=3,
                                    padded_shape=[P, 512], name=f"thv_{t}")
                    for k in range(KC):
                        nc.tensor.matmul(
                            th_qk[:],
                            lhsT=xts[k][:, tsl],
                            rhs=w_t[:, k, 0:2 * EG],
                            start=(k == 0), stop=(k == KC - 1),
                        )
                        nc.tensor.matmul(
                            th_v[:],
                            lhsT=xts[k][:, tsl],
                            rhs=w_t[:, k, 2 * EG:3 * EG],
                            start=(k == 0), stop=(k == KC - 1),
                        )
                    c = cp.tile([P, 3 * EG], F16, tag="c", name=f"c{t}")
                    nc.scalar.activation(
                        c[:, 0:2 * EG], th_qk[:], AF.Sin, bias=hp[:])
                    nc.scalar.activation(
                        c[:, 2 * EG:3 * EG], th_v[:], AF.Sin, bias=hp[:])
                    zq_s = zsp.tile([P, EG], F16, tag="zq", bufs=3,
                                    name=f"zqs{t}")
                    zk_s = zsp.tile([P, EG], F16, tag="zk", bufs=3,
                                    name=f"zks{t}")
                    # q,k scans first: they feed the transposes (critical
                    # path); v scans are only consumed in phase B
                    for h in range(HG):
                        dsl = slice(h * D, (h + 1) * D)
                        nc.vector.tensor_tensor_scan(
                            zq_s[:, dsl], c[:, dsl], ones[:], 1.0,
                            ALU.mult, ALU.bypass)
                        nc.vector.tensor_tensor_scan(
                            zk_s[:, dsl], c[:, EG + h * D:EG + (h + 1) * D],
                            ones[:], 1.0, ALU.mult, ALU.bypass)
                    for h in range(HG):
                        nc.vector.tensor_tensor_scan(
                            vzts[t][:, 2 * h + 1, :],
                            c[:, 2 * EG + h * D:2 * EG + (h + 1) * D],
                            ones[:], 1.0, ALU.mult, ALU.bypass)
                    if _DEBUG and t == 0:
                        nc.sync.dma_start(out=dbg["dbg_c0"][:], in_=c[:])
                        nc.sync.dma_start(out=dbg["dbg_zqs0"][:], in_=zq_s[:])
                        nc.sync.dma_start(out=dbg["dbg_zks0"][:], in_=zk_s[:])
                    trans_q.append((t, zq_s, zk_s))
                if t > 0:
                    tp, zq_s, zk_s = trans_q[t - 1]
                    tsl = slice(tp * P, (tp + 1) * P)
                    for m in range(2):
                        msl = slice(m * P, (m + 1) * P)
                        for ci, (src, dst) in enumerate(
                                ((zq_s, zqT[m]), (zk_s, zkT[m]))):
                            pt = psT.tile([P, P], F16, tag="pt", bufs=2,
                                          padded_shape=[P, 1024],
                                          name=f"pt{tp}{m}")
                            nc.tensor.transpose(pt[:], src[:, msl], id_t[:])
                            # GPSIMD cannot read PSUM; Copy is in every ACT
                            # table so it costs no table switch. The last two
                            # tiles' copies gate the phase A->B pool handoff:
                            # split them across both engines to halve that
                            # serial chain.
                            if tp >= NT - 2 and (2 * m + ci) % 2 == 1:
                                nc.vector.tensor_copy(out=dst[:, tsl],
                                                      in_=pt[:])
                            else:
                                nc.scalar.copy(out=dst[:, tsl], in_=pt[:])

        if _DEBUG:
            nc.sync.dma_start(out=dbg["dbg_zqT0"][:], in_=zqT[0][:])
            nc.sync.dma_start(out=dbg["dbg_zkT0"][:], in_=zkT[0][:])
            nc.sync.dma_start(
                out=dbg["dbg_vz0"][:],
                in_=vzts[0][:].rearrange("p a d -> p (a d)"))

        # ---------------- Phase B (attention + fused final projection) ----
        with tc.tile_pool(name="oz", bufs=1) as ozp:
            oz_tiles = [ozp.tile([P, S], F16, tag=f"oz{m}", name=f"oz{m}")
                        for m in range(2)]
            _attention(tc, oz_tiles, zqT, zkT, vzts, wc_t, yT, dbg)
            if _DEBUG:
                nc.sync.dma_start(out=dbg["dbg_oz0"][:], in_=oz_tiles[0][:])


def build_bass():
    nc = bacc.Bacc(None, target_bir_lowering=False)
    xT = nc.dram_tensor("xT", [E, S], F16, kind="ExternalInput")
    wT = nc.dram_tensor("wT", [E, 3 * EG], F16, kind="ExternalInput")
    wcT = nc.dram_tensor("wcT", [EG, E], F16, kind="ExternalInput")
    ident = nc.dram_tensor("ident", [P, P], F16, kind="ExternalInput")
    yT = nc.dram_tensor("yT", [E, S], F16, kind="ExternalOutput")
    dbg = {}
    if _DEBUG:
        for nm, shp, dt in (("dbg_zqT0", [P, S], F16), ("dbg_zkT0", [P, S], F16),
                            ("dbg_vz0", [P, 8 * D], F16),
                            ("dbg_oz0", [P, S], F16),
                            ("dbg_c0", [P, 3 * EG], F16),
                            ("dbg_zqs0", [P, EG], F16),
                            ("dbg_zks0", [P, EG], F16),
                            ("dbg_ex0", [P, 512], F16)):
            dbg[nm] = nc.dram_tensor(nm, shp, dt, kind="ExternalOutput")[:]
    with tile.TileContext(nc) as tc:
        _build_body(tc, xT[:], wT[:], wcT[:], ident[:], yT[:], dbg)
    nc.finalize()
    return nc


_NC_CACHE = None


def _get_nc():
    global _NC_CACHE
    if _NC_CACHE is None:
        _NC_CACHE = build_bass()
    return _NC_CACHE


def kernel(x, Wq, Wk, Wv, Wc, bc, **kw):
    x = np.asarray(x, np.float32)
    ident = np.eye(P, dtype=np.float16)
    in_maps = []
    for c in range(NCORES):
        b, g = divmod(c, NCORES // B)
        sl = slice(g * EG, (g + 1) * EG)
        wqkv = np.concatenate(
            [np.asarray(Wq)[sl, :].T, np.asarray(Wk)[sl, :].T,
             np.asarray(Wv)[sl, :].T], axis=1).astype(np.float16)
        in_maps.append({
            "xT": np.ascontiguousarray(np.asarray(x[b]).T.astype(np.float16)),
            "wT": np.ascontiguousarray(wqkv),
            "wcT": np.ascontiguousarray(
                np.asarray(Wc)[:, sl].T.astype(np.float16)),
            "ident": ident,
        })
    nc = _get_nc()
    res = run_bass_kernel_spmd(
        nc, in_maps, core_ids=list(range(NCORES)),
        trace=bool(int(os.environ.get("QK_TRACE", "0"))),
    )
    y = np.zeros((B, S, E), np.float32)
    for c in range(NCORES):
        b = c // (NCORES // B)
        y[b] += res.results[c]["yT"].astype(np.float32).T
    y += np.asarray(bc, np.float32)
    globals()["_LAST_RESULT"] = res
    return y
